# revision 19
# baseline (speedup 1.0000x reference)
"""Trainium2 Bass kernel for 16-head MHA (d_model=1024, batch 4, seq 2048).

Sharding: batch (4) x head-group (2) across 8 NeuronCores. Each core computes
one batch sample's attention for 8 of the 16 heads plus its partial output
projection; the host sums the two partial outputs per sample and adds the
bias terms.

Per-core dataflow (all matmul contractions run on the partition axis):
  q^T/k^T = WqT'.T @ x^T   (fp16, out [o, t] with heads on partitions)
  v       = x^T.T @ WvT    (fp16 matmul, bf16 store, out [t, o], with a ones
                            column appended per head for fused softmax rowsums)
  E^T     = exp(k^T_h.T @ q^T_h)  (scores accumulate f32 in PSUM, exp on ACT,
                                   bf16 store; no max-subtraction needed: the
                                   reference applies no 1/sqrt(d) scaling and
                                   scores stay < ~50, well within f32/bf16 range)
  att_h   = (V_h|1).T @ E^T_h     -> rows 0:64 raw attention, row 64 rowsum
  att^T   = att_h * recip(rowsum) (DVE + gpsimd broadcast, off critical path)
  y^T     = WoT'.T @ att^T        (fp32 out)

fp16 is used for the whole q/k/scores path: bf16's 8-bit mantissa gives score
errors ~0.05 which exp() amplifies to ~2e-2 output error; fp16 keeps it ~3e-3.

Loop structure: query-halves outermost so the output projection for half 0
overlaps the attention of half 1; projections pair two moving streams per
stationary load to hide weight-switch bubbles; the kc loop is
software-pipelined (S1(kc+1) issues before S2(kc)) so the PE never waits for
the ACT exp.
"""

from contextlib import ExitStack

import numpy as np

import concourse.bacc as bacc
import concourse.mybir as mybir
import concourse.tile as tile
from concourse.bass_utils import run_bass_kernel_spmd

F32 = mybir.dt.float32
F16 = mybir.dt.float16
BF16 = mybir.dt.bfloat16

D = 1024          # d_model
HD = 64           # head dim
NH_CORE = 8       # heads per core
OC = NH_CORE * HD # per-core q/k/v output dims (512)
N_CORES = 8
NI = D // 128     # contraction chunks for projections
NOC = OC // 128   # o-chunks (head pairs)
NDC = D // 128    # output-dim chunks for the final projection


def build_kernel(S=2048):
    nc = bacc.Bacc("TRN2", target_bir_lowering=False, debug=False)

    xq_d = nc.dram_tensor("xqT", (D, S), F16, kind="ExternalInput")
    xk_d = nc.dram_tensor("xkT", (D, S), F16, kind="ExternalInput")
    xv_d = nc.dram_tensor("xvT", (D, S), F16, kind="ExternalInput")
    wq_d = nc.dram_tensor("WqT", (D, OC), F16, kind="ExternalInput")
    wk_d = nc.dram_tensor("WkT", (D, OC), F16, kind="ExternalInput")
    wv_d = nc.dram_tensor("WvT", (D, OC), F16, kind="ExternalInput")
    wo_d = nc.dram_tensor("WoT", (OC, D), F16, kind="ExternalInput")
    bq_d = nc.dram_tensor("bq", (OC,), F32, kind="ExternalInput")
    bk_d = nc.dram_tensor("bk", (OC,), F32, kind="ExternalInput")
    y_d = nc.dram_tensor("yT", (D, S), F32, kind="ExternalOutput")

    NKC = S // 128            # key chunks
    HF = S // 2               # query-half size (exp/psum granularity)
    QC = min(512, HF)         # matmul moving size
    NQC = HF // QC
    NTQ = S // QC             # 512-col chunks over the full sequence
    VT = min(256, S)          # xv chunk width (2 stationary tiles per chunk)

    Exp = mybir.ActivationFunctionType.Exp
    Mult = mybir.AluOpType.mult

    with tile.TileContext(nc) as tc, ExitStack() as ctx:
        wpool = ctx.enter_context(tc.tile_pool(name="w", bufs=1))
        xpool = ctx.enter_context(tc.tile_pool(name="x", bufs=1))
        spool = ctx.enter_context(tc.tile_pool(name="seq", bufs=1))
        epool = ctx.enter_context(tc.tile_pool(name="e", bufs=2))
        evpool = ctx.enter_context(tc.tile_pool(name="ev", bufs=3))
        npool = ctx.enter_context(tc.tile_pool(name="nrm", bufs=2))
        pjpool = ctx.enter_context(tc.tile_pool(name="pj", bufs=2, space="PSUM"))
        s1pool = ctx.enter_context(tc.tile_pool(name="s1", bufs=2, space="PSUM"))
        accpool = ctx.enter_context(tc.tile_pool(name="acc", bufs=2, space="PSUM"))

        # ---- resident weights / biases / x slabs ----
        wq_sb = wpool.tile([128, NI, OC], F16, tag="wq")
        wk_sb = wpool.tile([128, NI, OC], F16, tag="wk")
        wv_sb = wpool.tile([128, NI, OC], F16, tag="wv")
        wo_sb = wpool.tile([128, NOC, D], F16, tag="wo")
        nc.sync.dma_start(out=wv_sb, in_=wv_d.ap().rearrange("(ic p) o -> p ic o", p=128))
        nc.sync.dma_start(out=wq_sb, in_=wq_d.ap().rearrange("(ic p) o -> p ic o", p=128))
        nc.sync.dma_start(out=wk_sb, in_=wk_d.ap().rearrange("(ic p) o -> p ic o", p=128))
        nc.sync.dma_start(out=wo_sb, in_=wo_d.ap().rearrange("(oc p) d -> p oc d", p=128))
        bq_sb = wpool.tile([128, NOC], F32, tag="bq")
        bk_sb = wpool.tile([128, NOC], F32, tag="bk")
        nc.sync.dma_start(out=bq_sb, in_=bq_d.ap().rearrange("(c p) -> p c", p=128))
        nc.sync.dma_start(out=bk_sb, in_=bk_d.ap().rearrange("(c p) -> p c", p=128))

        xq_sb = xpool.tile([128, NI, S], F16, tag="xq")
        xk_sb = xpool.tile([128, NI, S], F16, tag="xk")
        nc.sync.dma_start(out=xq_sb, in_=xq_d.ap().rearrange("(ic p) t -> p ic t", p=128))
        nc.sync.dma_start(out=xk_sb, in_=xk_d.ap().rearrange("(ic p) t -> p ic t", p=128))

        # ---- per-sequence slabs ----
        v_sb = spool.tile([128, NKC, NH_CORE * (HD + 1)], BF16, tag="v")
        qT_sb = spool.tile([128, NOC, S], F16, tag="qT")
        kT_sb = spool.tile([128, NOC, S], F16, tag="kT")
        att_sb = [
            spool.tile([128, NOC, HF], F16, tag=f"att{hf}", name=f"att{hf}")
            for hf in range(2)
        ]

        # ---- phase 0: v projection (v[t, o] layout, 65-stride + ones col) ----
        # xv streams in [128, VT] chunks so the big xq/xk DMAs overlap phase 0
        NVG = S // VT
        NVT = VT // 128
        for tcg in range(NVG):
            xvc = [
                xpool.tile([128, VT], F16, tag="xvc", bufs=16, name=f"xvc{tcg}_{ic}")
                for ic in range(NI)
            ]
            for ic in range(NI):
                nc.sync.dma_start(
                    out=xvc[ic],
                    in_=xv_d.ap()[ic * 128:(ic + 1) * 128, tcg * VT:(tcg + 1) * VT],
                )
            vps = [
                pjpool.tile([128, OC], F32, tag="pj", name=f"vps{tcg}_{j}")
                for j in range(NVT)
            ]
            for ic in range(NI):
                for j in range(NVT):
                    nc.tensor.matmul(
                        vps[j][:, :],
                        xvc[ic][:, j * 128:(j + 1) * 128],
                        wv_sb[:, ic, :],
                        start=(ic == 0), stop=(ic == NI - 1),
                    )
            for j in range(NVT):
                tci = tcg * NVT + j
                vv = v_sb[:, tci, :].rearrange("p (h c) -> p h c", h=NH_CORE)
                nc.vector.tensor_copy(
                    out=vv[:, :, 0:HD],
                    in_=vps[j][:, :].rearrange("p (h c) -> p h c", c=HD),
                )
                nc.vector.memset(vv[:, :, HD:HD + 1], 1.0)

        # ---- query-half outer loop: projections, attention, y-projection ----
        for hf in range(2):
            for hp in range(NOC):
                if hf == 0:
                    for pi, (w_sb, x_sb, b_sb, dst) in enumerate((
                        (wq_sb, xq_sb, bq_sb, qT_sb),
                        (wk_sb, xk_sb, bk_sb, kT_sb),
                    )):
                        for tp in range(NTQ // 2):
                            pps = [
                                pjpool.tile([128, QC], F32, tag="pj",
                                            name=f"pj{hp}_{pi}_{tp}_{j}")
                                for j in range(2)
                            ]
                            for ic in range(NI):
                                for j in range(2):
                                    tq = tp * 2 + j
                                    nc.tensor.matmul(
                                        pps[j][:, :],
                                        w_sb[:, ic, hp * 128:(hp + 1) * 128],
                                        x_sb[:, ic, tq * QC:(tq + 1) * QC],
                                        start=(ic == 0), stop=(ic == NI - 1),
                                    )
                            for j in range(2):
                                tq = tp * 2 + j
                                nc.vector.tensor_scalar_add(
                                    out=dst[:, hp, tq * QC:(tq + 1) * QC],
                                    in0=pps[j][:, :],
                                    scalar1=b_sb[:, hp:hp + 1],
                                )

                for hl in range(2):
                    h = 2 * hp + hl
                    off = hl * 64
                    vslice = slice(h * (HD + 1), (h + 1) * (HD + 1))
                    accs = [
                        accpool.tile([65, QC], F32, tag="acc", name=f"acc{h}_{hf}_{i}")
                        for i in range(NQC)
                    ]

                    def s2(e, kc):
                        for tq in range(NQC):
                            nc.tensor.matmul(
                                accs[tq][:, :],
                                v_sb[:, kc, vslice],
                                e[:, tq * QC:(tq + 1) * QC],
                                start=(kc == 0), stop=(kc == NKC - 1),
                            )

                    prev = None
                    for kc in range(NKC):
                        s1 = s1pool.tile([128, HF], F32, tag="s1", name=f"s1_{h}_{hf}_{kc}")
                        for tq in range(NQC):
                            nc.tensor.matmul(
                                s1[:, tq * QC:(tq + 1) * QC],
                                kT_sb[off:off + 64, hp, kc * 128:(kc + 1) * 128],
                                qT_sb[off:off + 64, hp,
                                      hf * HF + tq * QC: hf * HF + (tq + 1) * QC],
                                start=True, stop=True,
                            )
                        e = epool.tile([128, HF], BF16, tag="e", name=f"e{h}_{hf}_{kc}")
                        nc.scalar.activation(out=e[:, :], in_=s1[:, :], func=Exp)
                        if prev is not None:
                            s2(*prev)
                        prev = (e, kc)
                    s2(*prev)

                    # free the acc banks first, then normalize from the copies
                    asbs = []
                    for tq in range(NQC):
                        asb = npool.tile([65, QC], F32, tag="accsb", bufs=2,
                                         name=f"asb{h}_{hf}_{tq}")
                        nc.vector.tensor_copy(out=asb[:, :], in_=accs[tq][:, :])
                        asbs.append(asb)
                    for tq in range(NQC):
                        asb = asbs[tq]
                        rt = npool.tile([1, QC], F32, tag="rtmp", bufs=2,
                                        name=f"rt{h}_{hf}_{tq}")
                        nc.vector.tensor_copy(out=rt[:, :], in_=asb[64:65, :])
                        nc.vector.reciprocal_approx_fast(out=rt[:, :], in_=rt[:, :])
                        bc = npool.tile([64, QC], F32, tag="bcast", bufs=2,
                                        name=f"bc{h}_{hf}_{tq}")
                        nc.gpsimd.partition_broadcast(out_ap=bc[:, :], in_ap=rt[:, :])
                        nc.vector.tensor_tensor(
                            out=att_sb[hf][off:off + 64, hp, tq * QC:(tq + 1) * QC],
                            in0=asb[0:64, :],
                            in1=bc[:, :],
                            op=Mult,
                        )

            # ---- output projection for this query half ----
            for dc in range(NDC):
                yps = [
                    pjpool.tile([128, QC], F32, tag="pj", name=f"yps{hf}_{dc}_{j}")
                    for j in range(NQC)
                ]
                for oc in range(NOC):
                    for j in range(NQC):
                        nc.tensor.matmul(
                            yps[j][:, :],
                            wo_sb[:, oc, dc * 128:(dc + 1) * 128],
                            att_sb[hf][:, oc, j * QC:(j + 1) * QC],
                            start=(oc == 0), stop=(oc == NOC - 1),
                        )
                for j in range(NQC):
                    y_sb = evpool.tile([128, QC], F32, tag="yev", name=f"yev{hf}_{dc}_{j}")
                    nc.vector.tensor_copy(out=y_sb[:, :], in_=yps[j][:, :])
                    nc.sync.dma_start(
                        out=y_d.ap()[dc * 128:(dc + 1) * 128,
                                     hf * HF + j * QC: hf * HF + (j + 1) * QC],
                        in_=y_sb[:, :],
                    )

    nc.compile()
    return nc


def make_in_maps(query, key, value, Wq, bq, Wk, bk, Wv, bv, Wo, bo):
    """Shard + lay out full inputs for the 8 cores: core = 2*n + g."""
    f16 = np.float16
    N = query.shape[0]
    per_g = {}
    for g in range(2):
        osl = slice(g * OC, (g + 1) * OC)
        per_g[g] = dict(
            WqT=np.ascontiguousarray(Wq[osl, :].T).astype(f16),
            WkT=np.ascontiguousarray(Wk[osl, :].T).astype(f16),
            WvT=np.ascontiguousarray(Wv[osl, :].T).astype(f16),
            WoT=np.ascontiguousarray(Wo[:, osl].T).astype(f16),
            bq=np.ascontiguousarray(bq[osl]).astype(np.float32),
            bk=np.ascontiguousarray(bk[osl]).astype(np.float32),
        )
    in_maps = []
    for n in range(N):
        xqT = np.ascontiguousarray(query[n].T).astype(f16)
        xkT = np.ascontiguousarray(key[n].T).astype(f16)
        xvT = np.ascontiguousarray(value[n].T).astype(f16)
        for g in range(2):
            m = dict(xqT=xqT, xkT=xkT, xvT=xvT)
            m.update(per_g[g])
            in_maps.append(m)
    return in_maps


_BUILT = None


def _get_built():
    global _BUILT
    if _BUILT is None:
        _BUILT = build_kernel(2048)
    return _BUILT


def kernel(query, key, value, Wq, bq, Wk, bk, Wv, bv, Wo, bo, _results=None):
    query = np.asarray(query, np.float32)
    key = np.asarray(key, np.float32)
    value = np.asarray(value, np.float32)
    Wq, bq = np.asarray(Wq, np.float32), np.asarray(bq, np.float32)
    Wk, bk = np.asarray(Wk, np.float32), np.asarray(bk, np.float32)
    Wv, bv = np.asarray(Wv, np.float32), np.asarray(bv, np.float32)
    Wo, bo = np.asarray(Wo, np.float32), np.asarray(bo, np.float32)

    N, S, _ = query.shape
    if _results is None:
        nc = _get_built()
        in_maps = make_in_maps(query, key, value, Wq, bq, Wk, bk, Wv, bv, Wo, bo)
        res = run_bass_kernel_spmd(nc, in_maps, list(range(N_CORES)))
        _results = res.results

    const = bv @ Wo.T + bo  # host-folded bias terms
    out = np.empty((N, S, D), np.float32)
    for n in range(N):
        yT = _results[2 * n]["yT"] + _results[2 * n + 1]["yT"]
        out[n] = yT.T + const
    return out


# revision 20
# speedup vs baseline: 1.0191x; 1.0191x over previous
"""Trainium2 Bass kernel for 16-head MHA (d_model=1024, batch 4, seq 2048).

Sharding: batch (4) x head-group (2) across 8 NeuronCores. Each core computes
one batch sample's attention for 8 of the 16 heads plus its partial output
projection; the host sums the two partial outputs per sample and adds the
bias terms.

Per-core dataflow (all matmul contractions run on the partition axis):
  q^T/k^T = WqT'.T @ x^T   (fp16, out [o, t] with heads on partitions)
  v       = x^T.T @ WvT    (fp16 matmul, bf16 store, out [t, o], with a ones
                            column appended per head for fused softmax rowsums)
  E^T     = exp(k^T_h.T @ q^T_h)  (scores accumulate f32 in PSUM, exp on ACT,
                                   bf16 store; no max-subtraction needed: the
                                   reference applies no 1/sqrt(d) scaling and
                                   scores stay < ~50, well within f32/bf16 range)
  att_h   = (V_h|1).T @ E^T_h     -> rows 0:64 raw attention, row 64 rowsum
  att^T   = att_h * recip(rowsum) (DVE + gpsimd broadcast, off critical path)
  y^T     = WoT'.T @ att^T        (fp32 out)

fp16 is used for the whole q/k/scores path: bf16's 8-bit mantissa gives score
errors ~0.05 which exp() amplifies to ~2e-2 output error; fp16 keeps it ~3e-3.

Loop structure: query-halves outermost so the output projection for half 0
overlaps the attention of half 1; projections pair two moving streams per
stationary load to hide weight-switch bubbles; the kc loop is
software-pipelined (S1(kc+1) issues before S2(kc)) so the PE never waits for
the ACT exp.
"""

from contextlib import ExitStack

import numpy as np

import concourse.bacc as bacc
import concourse.mybir as mybir
import concourse.tile as tile
from concourse.bass_utils import run_bass_kernel_spmd

F32 = mybir.dt.float32
F16 = mybir.dt.float16
BF16 = mybir.dt.bfloat16

D = 1024          # d_model
HD = 64           # head dim
NH_CORE = 8       # heads per core
OC = NH_CORE * HD # per-core q/k/v output dims (512)
N_CORES = 8
NI = D // 128     # contraction chunks for projections
NOC = OC // 128   # o-chunks (head pairs)
NDC = D // 128    # output-dim chunks for the final projection


def build_kernel(S=2048):
    nc = bacc.Bacc("TRN2", target_bir_lowering=False, debug=False)

    xq_d = nc.dram_tensor("xqT", (D, S), F16, kind="ExternalInput")
    xk_d = nc.dram_tensor("xkT", (D, S), F16, kind="ExternalInput")
    xv_d = nc.dram_tensor("xvT", (D, S), F16, kind="ExternalInput")
    wq_d = nc.dram_tensor("WqT", (D, OC), F16, kind="ExternalInput")
    wk_d = nc.dram_tensor("WkT", (D, OC), F16, kind="ExternalInput")
    wv_d = nc.dram_tensor("WvT", (D, OC), F16, kind="ExternalInput")
    wo_d = nc.dram_tensor("WoT", (OC, D), F16, kind="ExternalInput")
    bq_d = nc.dram_tensor("bq", (OC,), F32, kind="ExternalInput")
    bk_d = nc.dram_tensor("bk", (OC,), F32, kind="ExternalInput")
    y_d = nc.dram_tensor("yT", (D, S), F32, kind="ExternalOutput")

    NKC = S // 128            # key chunks
    HF = S // 2               # query-half size (exp/psum granularity)
    QC = min(512, HF)         # matmul moving size
    NQC = HF // QC
    NTQ = S // QC             # 512-col chunks over the full sequence
    VT = min(256, S)          # xv chunk width (2 stationary tiles per chunk)

    Exp = mybir.ActivationFunctionType.Exp
    Mult = mybir.AluOpType.mult

    with tile.TileContext(nc) as tc, ExitStack() as ctx:
        wpool = ctx.enter_context(tc.tile_pool(name="w", bufs=1))
        xpool = ctx.enter_context(tc.tile_pool(name="x", bufs=1))
        spool = ctx.enter_context(tc.tile_pool(name="seq", bufs=1))
        epool = ctx.enter_context(tc.tile_pool(name="e", bufs=3))
        evpool = ctx.enter_context(tc.tile_pool(name="ev", bufs=2))
        npool = ctx.enter_context(tc.tile_pool(name="nrm", bufs=2))
        pjpool = ctx.enter_context(tc.tile_pool(name="pj", bufs=2, space="PSUM"))
        s1pool = ctx.enter_context(tc.tile_pool(name="s1", bufs=2, space="PSUM"))
        accpool = ctx.enter_context(tc.tile_pool(name="acc", bufs=2, space="PSUM"))

        # ---- resident weights / biases / x slabs ----
        wq_sb = wpool.tile([128, NI, OC], F16, tag="wq")
        wk_sb = wpool.tile([128, NI, OC], F16, tag="wk")
        wv_sb = wpool.tile([128, NI, OC], F16, tag="wv")
        wo_sb = wpool.tile([128, NOC, D], F16, tag="wo")
        nc.sync.dma_start(out=wv_sb, in_=wv_d.ap().rearrange("(ic p) o -> p ic o", p=128))
        nc.sync.dma_start(out=wq_sb, in_=wq_d.ap().rearrange("(ic p) o -> p ic o", p=128))
        nc.sync.dma_start(out=wk_sb, in_=wk_d.ap().rearrange("(ic p) o -> p ic o", p=128))
        nc.sync.dma_start(out=wo_sb, in_=wo_d.ap().rearrange("(oc p) d -> p oc d", p=128))
        bq_sb = wpool.tile([128, NOC], F32, tag="bq")
        bk_sb = wpool.tile([128, NOC], F32, tag="bk")
        nc.sync.dma_start(out=bq_sb, in_=bq_d.ap().rearrange("(c p) -> p c", p=128))
        nc.sync.dma_start(out=bk_sb, in_=bk_d.ap().rearrange("(c p) -> p c", p=128))

        xq_sb = xpool.tile([128, NI, S], F16, tag="xq")
        xk_sb = xpool.tile([128, NI, S], F16, tag="xk")
        nc.sync.dma_start(out=xq_sb, in_=xq_d.ap().rearrange("(ic p) t -> p ic t", p=128))
        nc.sync.dma_start(out=xk_sb, in_=xk_d.ap().rearrange("(ic p) t -> p ic t", p=128))

        # ---- per-sequence slabs ----
        v_sb = spool.tile([128, NKC, NH_CORE * (HD + 1)], BF16, tag="v")
        qT_sb = spool.tile([128, NOC, S], F16, tag="qT")
        kT_sb = spool.tile([128, NOC, S], F16, tag="kT")
        att_sb = [
            spool.tile([128, NOC, HF], F16, tag=f"att{hf}", name=f"att{hf}")
            for hf in range(2)
        ]

        # ---- phase 0: v projection (v[t, o] layout, 65-stride + ones col) ----
        # xv streams in [128, VT] chunks so the big xq/xk DMAs overlap phase 0
        NVG = S // VT
        NVT = VT // 128
        for tcg in range(NVG):
            xvc = [
                xpool.tile([128, VT], F16, tag="xvc", bufs=16, name=f"xvc{tcg}_{ic}")
                for ic in range(NI)
            ]
            for ic in range(NI):
                nc.sync.dma_start(
                    out=xvc[ic],
                    in_=xv_d.ap()[ic * 128:(ic + 1) * 128, tcg * VT:(tcg + 1) * VT],
                )
            vps = [
                pjpool.tile([128, OC], F32, tag="pj", name=f"vps{tcg}_{j}")
                for j in range(NVT)
            ]
            for ic in range(NI):
                for j in range(NVT):
                    nc.tensor.matmul(
                        vps[j][:, :],
                        xvc[ic][:, j * 128:(j + 1) * 128],
                        wv_sb[:, ic, :],
                        start=(ic == 0), stop=(ic == NI - 1),
                    )
            for j in range(NVT):
                tci = tcg * NVT + j
                vv = v_sb[:, tci, :].rearrange("p (h c) -> p h c", h=NH_CORE)
                nc.vector.tensor_copy(
                    out=vv[:, :, 0:HD],
                    in_=vps[j][:, :].rearrange("p (h c) -> p h c", c=HD),
                )
                nc.vector.memset(vv[:, :, HD:HD + 1], 1.0)

        # ---- query-half outer loop: projections, attention, y-projection ----
        for hf in range(2):
            for hp in range(NOC):
                if hf == 0:
                    for pi, (w_sb, x_sb, b_sb, dst) in enumerate((
                        (wq_sb, xq_sb, bq_sb, qT_sb),
                        (wk_sb, xk_sb, bk_sb, kT_sb),
                    )):
                        for tp in range(NTQ // 2):
                            pps = [
                                pjpool.tile([128, QC], F32, tag="pj",
                                            name=f"pj{hp}_{pi}_{tp}_{j}")
                                for j in range(2)
                            ]
                            for ic in range(NI):
                                for j in range(2):
                                    tq = tp * 2 + j
                                    nc.tensor.matmul(
                                        pps[j][:, :],
                                        w_sb[:, ic, hp * 128:(hp + 1) * 128],
                                        x_sb[:, ic, tq * QC:(tq + 1) * QC],
                                        start=(ic == 0), stop=(ic == NI - 1),
                                    )
                            for j in range(2):
                                tq = tp * 2 + j
                                nc.vector.tensor_scalar_add(
                                    out=dst[:, hp, tq * QC:(tq + 1) * QC],
                                    in0=pps[j][:, :],
                                    scalar1=b_sb[:, hp:hp + 1],
                                )

                for hl in range(2):
                    h = 2 * hp + hl
                    off = hl * 64
                    vslice = slice(h * (HD + 1), (h + 1) * (HD + 1))
                    accs = [
                        accpool.tile([65, QC], F32, tag="acc", name=f"acc{h}_{hf}_{i}")
                        for i in range(NQC)
                    ]

                    def s2(e, kc):
                        for tq in range(NQC):
                            nc.tensor.matmul(
                                accs[tq][:, :],
                                v_sb[:, kc, vslice],
                                e[:, tq * QC:(tq + 1) * QC],
                                start=(kc == 0), stop=(kc == NKC - 1),
                            )

                    prev = None
                    for kc in range(NKC):
                        s1 = s1pool.tile([128, HF], F32, tag="s1", name=f"s1_{h}_{hf}_{kc}")
                        for tq in range(NQC):
                            nc.tensor.matmul(
                                s1[:, tq * QC:(tq + 1) * QC],
                                kT_sb[off:off + 64, hp, kc * 128:(kc + 1) * 128],
                                qT_sb[off:off + 64, hp,
                                      hf * HF + tq * QC: hf * HF + (tq + 1) * QC],
                                start=True, stop=True,
                            )
                        e = epool.tile([128, HF], BF16, tag="e", name=f"e{h}_{hf}_{kc}")
                        nc.scalar.activation(out=e[:, :], in_=s1[:, :], func=Exp)
                        if prev is not None:
                            s2(*prev)
                        prev = (e, kc)
                    s2(*prev)

                    # free the acc banks first, then normalize from the copies
                    asbs = []
                    for tq in range(NQC):
                        asb = npool.tile([65, QC], F32, tag="accsb", bufs=2,
                                         name=f"asb{h}_{hf}_{tq}")
                        nc.vector.tensor_copy(out=asb[:, :], in_=accs[tq][:, :])
                        asbs.append(asb)
                    for tq in range(NQC):
                        asb = asbs[tq]
                        rt = npool.tile([1, QC], F32, tag="rtmp", bufs=2,
                                        name=f"rt{h}_{hf}_{tq}")
                        nc.vector.tensor_copy(out=rt[:, :], in_=asb[64:65, :])
                        nc.vector.reciprocal_approx_fast(out=rt[:, :], in_=rt[:, :])
                        bc = npool.tile([64, QC], F32, tag="bcast", bufs=2,
                                        name=f"bc{h}_{hf}_{tq}")
                        nc.gpsimd.partition_broadcast(out_ap=bc[:, :], in_ap=rt[:, :])
                        nc.vector.tensor_tensor(
                            out=att_sb[hf][off:off + 64, hp, tq * QC:(tq + 1) * QC],
                            in0=asb[0:64, :],
                            in1=bc[:, :],
                            op=Mult,
                        )

            # ---- output projection for this query half ----
            for dc in range(NDC):
                yps = [
                    pjpool.tile([128, QC], F32, tag="pj", name=f"yps{hf}_{dc}_{j}")
                    for j in range(NQC)
                ]
                for oc in range(NOC):
                    for j in range(NQC):
                        nc.tensor.matmul(
                            yps[j][:, :],
                            wo_sb[:, oc, dc * 128:(dc + 1) * 128],
                            att_sb[hf][:, oc, j * QC:(j + 1) * QC],
                            start=(oc == 0), stop=(oc == NOC - 1),
                        )
                for j in range(NQC):
                    y_sb = evpool.tile([128, QC], F32, tag="yev", name=f"yev{hf}_{dc}_{j}")
                    nc.vector.tensor_copy(out=y_sb[:, :], in_=yps[j][:, :])
                    nc.sync.dma_start(
                        out=y_d.ap()[dc * 128:(dc + 1) * 128,
                                     hf * HF + j * QC: hf * HF + (j + 1) * QC],
                        in_=y_sb[:, :],
                    )

    nc.compile()
    return nc


def make_in_maps(query, key, value, Wq, bq, Wk, bk, Wv, bv, Wo, bo):
    """Shard + lay out full inputs for the 8 cores: core = 2*n + g."""
    f16 = np.float16
    N = query.shape[0]
    per_g = {}
    for g in range(2):
        osl = slice(g * OC, (g + 1) * OC)
        per_g[g] = dict(
            WqT=np.ascontiguousarray(Wq[osl, :].T).astype(f16),
            WkT=np.ascontiguousarray(Wk[osl, :].T).astype(f16),
            WvT=np.ascontiguousarray(Wv[osl, :].T).astype(f16),
            WoT=np.ascontiguousarray(Wo[:, osl].T).astype(f16),
            bq=np.ascontiguousarray(bq[osl]).astype(np.float32),
            bk=np.ascontiguousarray(bk[osl]).astype(np.float32),
        )
    in_maps = []
    for n in range(N):
        xqT = np.ascontiguousarray(query[n].T).astype(f16)
        xkT = np.ascontiguousarray(key[n].T).astype(f16)
        xvT = np.ascontiguousarray(value[n].T).astype(f16)
        for g in range(2):
            m = dict(xqT=xqT, xkT=xkT, xvT=xvT)
            m.update(per_g[g])
            in_maps.append(m)
    return in_maps


_BUILT = None


def _get_built():
    global _BUILT
    if _BUILT is None:
        _BUILT = build_kernel(2048)
    return _BUILT


def kernel(query, key, value, Wq, bq, Wk, bk, Wv, bv, Wo, bo, _results=None):
    query = np.asarray(query, np.float32)
    key = np.asarray(key, np.float32)
    value = np.asarray(value, np.float32)
    Wq, bq = np.asarray(Wq, np.float32), np.asarray(bq, np.float32)
    Wk, bk = np.asarray(Wk, np.float32), np.asarray(bk, np.float32)
    Wv, bv = np.asarray(Wv, np.float32), np.asarray(bv, np.float32)
    Wo, bo = np.asarray(Wo, np.float32), np.asarray(bo, np.float32)

    N, S, _ = query.shape
    if _results is None:
        nc = _get_built()
        in_maps = make_in_maps(query, key, value, Wq, bq, Wk, bk, Wv, bv, Wo, bo)
        res = run_bass_kernel_spmd(nc, in_maps, list(range(N_CORES)))
        _results = res.results

    const = bv @ Wo.T + bo  # host-folded bias terms
    out = np.empty((N, S, D), np.float32)
    for n in range(N):
        yT = _results[2 * n]["yT"] + _results[2 * n + 1]["yT"]
        out[n] = yT.T + const
    return out


# revision 24
# speedup vs baseline: 1.0325x; 1.0131x over previous
"""Trainium2 Bass kernel for 16-head MHA (d_model=1024, batch 4, seq 2048).

Sharding: batch (4) x head-group (2) across 8 NeuronCores. Each core computes
one batch sample's attention for 8 of the 16 heads plus its partial output
projection; the host sums the two partial outputs per sample and adds the
bias terms.

Per-core dataflow (all matmul contractions run on the partition axis):
  q^T/k^T = WqT'.T @ x^T   (fp16, out [o, t] with heads on partitions)
  v       = x^T.T @ WvT    (fp16 matmul, bf16 store, out [t, o], with a ones
                            column appended per head for fused softmax rowsums)
  E^T     = exp(k^T_h.T @ q^T_h)  (scores accumulate f32 in PSUM, exp on ACT,
                                   bf16 store; no max-subtraction needed: the
                                   reference applies no 1/sqrt(d) scaling and
                                   scores stay < ~50, well within f32/bf16 range)
  att_h   = (V_h|1).T @ E^T_h     -> rows 0:64 raw attention, row 64 rowsum
  att^T   = att_h * recip(rowsum) (DVE + gpsimd broadcast, off critical path)
  y^T     = WoT'.T @ att^T        (fp32 out)

fp16 is used for the whole q/k/scores path: bf16's 8-bit mantissa gives score
errors ~0.05 which exp() amplifies to ~2e-2 output error; fp16 keeps it ~3e-3.

Loop structure: query-halves outermost so the output projection for half 0
overlaps the attention of half 1; projections pair two moving streams per
stationary load to hide weight-switch bubbles; the kc loop is
software-pipelined (S1(kc+1) issues before S2(kc)) so the PE never waits for
the ACT exp.
"""

from contextlib import ExitStack

import numpy as np

import concourse.bacc as bacc
import concourse.mybir as mybir
import concourse.tile as tile
from concourse.bass_utils import run_bass_kernel_spmd

F32 = mybir.dt.float32
F16 = mybir.dt.float16
BF16 = mybir.dt.bfloat16

D = 1024          # d_model
HD = 64           # head dim
NH_CORE = 8       # heads per core
OC = NH_CORE * HD # per-core q/k/v output dims (512)
N_CORES = 8
NI = D // 128     # contraction chunks for projections
NOC = OC // 128   # o-chunks (head pairs)
NDC = D // 128    # output-dim chunks for the final projection


def build_kernel(S=2048):
    nc = bacc.Bacc("TRN2", target_bir_lowering=False, debug=False)

    xq_d = nc.dram_tensor("xqT", (D, S), F16, kind="ExternalInput")
    xk_d = nc.dram_tensor("xkT", (D, S), F16, kind="ExternalInput")
    xv_d = nc.dram_tensor("xvT", (D, S), F16, kind="ExternalInput")
    wq_d = nc.dram_tensor("WqT", (D, OC), F16, kind="ExternalInput")
    wk_d = nc.dram_tensor("WkT", (D, OC), F16, kind="ExternalInput")
    wv_d = nc.dram_tensor("WvT", (D, OC), F16, kind="ExternalInput")
    wo_d = nc.dram_tensor("WoT", (OC, D), F16, kind="ExternalInput")
    bq_d = nc.dram_tensor("bq", (OC,), F32, kind="ExternalInput")
    bk_d = nc.dram_tensor("bk", (OC,), F32, kind="ExternalInput")
    y_d = nc.dram_tensor("yT", (D, S), F32, kind="ExternalOutput")

    NKC = S // 128            # key chunks
    HF = S // 2               # query-half size (exp/psum granularity)
    QC = min(512, HF)         # matmul moving size
    NQC = HF // QC
    NTQ = S // QC             # 512-col chunks over the full sequence
    VT = min(256, S)          # xv chunk width (2 stationary tiles per chunk)

    Exp = mybir.ActivationFunctionType.Exp
    Mult = mybir.AluOpType.mult

    with tile.TileContext(nc) as tc, ExitStack() as ctx:
        wpool = ctx.enter_context(tc.tile_pool(name="w", bufs=1))
        xpool = ctx.enter_context(tc.tile_pool(name="x", bufs=1))
        spool = ctx.enter_context(tc.tile_pool(name="seq", bufs=1))
        epool = ctx.enter_context(tc.tile_pool(name="e", bufs=3))
        evpool = ctx.enter_context(tc.tile_pool(name="ev", bufs=2))
        npool = ctx.enter_context(tc.tile_pool(name="nrm", bufs=2))
        pjpool = ctx.enter_context(tc.tile_pool(name="pj", bufs=2, space="PSUM"))
        s1pool = ctx.enter_context(tc.tile_pool(name="s1", bufs=2, space="PSUM"))
        accpool = ctx.enter_context(tc.tile_pool(name="acc", bufs=2, space="PSUM"))

        # ---- resident weights / biases / x slabs ----
        wq_sb = wpool.tile([128, NI, OC], F16, tag="wq")
        wk_sb = wpool.tile([128, NI, OC], F16, tag="wk")
        wv_sb = wpool.tile([128, NI, OC], F16, tag="wv")
        wo_sb = wpool.tile([128, NOC, D], F16, tag="wo")
        nc.sync.dma_start(out=wv_sb, in_=wv_d.ap().rearrange("(ic p) o -> p ic o", p=128))
        nc.sync.dma_start(out=wq_sb, in_=wq_d.ap().rearrange("(ic p) o -> p ic o", p=128))
        nc.sync.dma_start(out=wk_sb, in_=wk_d.ap().rearrange("(ic p) o -> p ic o", p=128))
        nc.sync.dma_start(out=wo_sb, in_=wo_d.ap().rearrange("(oc p) d -> p oc d", p=128))
        bq_sb = wpool.tile([128, NOC], F32, tag="bq")
        bk_sb = wpool.tile([128, NOC], F32, tag="bk")
        nc.sync.dma_start(out=bq_sb, in_=bq_d.ap().rearrange("(c p) -> p c", p=128))
        nc.sync.dma_start(out=bk_sb, in_=bk_d.ap().rearrange("(c p) -> p c", p=128))

        xq_sb = xpool.tile([128, NI, S], F16, tag="xq")
        xk_sb = xpool.tile([128, NI, S], F16, tag="xk")
        nc.sync.dma_start(out=xq_sb, in_=xq_d.ap().rearrange("(ic p) t -> p ic t", p=128))
        nc.sync.dma_start(out=xk_sb, in_=xk_d.ap().rearrange("(ic p) t -> p ic t", p=128))

        # ---- per-sequence slabs ----
        v_sb = spool.tile([128, NKC, NH_CORE * (HD + 1)], BF16, tag="v")
        qT_sb = spool.tile([128, NOC, S], F16, tag="qT")
        kT_sb = spool.tile([128, NOC, S], F16, tag="kT")
        att_sb = [
            spool.tile([128, NOC, HF], F16, tag=f"att{hf}", name=f"att{hf}")
            for hf in range(2)
        ]

        # ---- phase 0: v projection (v[t, o] layout, 65-stride + ones col) ----
        # xv streams in [128, VT] chunks so the big xq/xk DMAs overlap phase 0
        NVG = S // VT
        NVT = VT // 128
        for tcg in range(NVG):
            xvc = [
                xpool.tile([128, VT], F16, tag="xvc", bufs=16, name=f"xvc{tcg}_{ic}")
                for ic in range(NI)
            ]
            for ic in range(NI):
                nc.sync.dma_start(
                    out=xvc[ic],
                    in_=xv_d.ap()[ic * 128:(ic + 1) * 128, tcg * VT:(tcg + 1) * VT],
                )
            vps = [
                pjpool.tile([128, OC], F32, tag="pj", name=f"vps{tcg}_{j}")
                for j in range(NVT)
            ]
            for ic in range(NI):
                for j in range(NVT):
                    nc.tensor.matmul(
                        vps[j][:, :],
                        xvc[ic][:, j * 128:(j + 1) * 128],
                        wv_sb[:, ic, :],
                        start=(ic == 0), stop=(ic == NI - 1),
                    )
            for j in range(NVT):
                tci = tcg * NVT + j
                vv = v_sb[:, tci, :].rearrange("p (h c) -> p h c", h=NH_CORE)
                nc.vector.tensor_copy(
                    out=vv[:, :, 0:HD],
                    in_=vps[j][:, :].rearrange("p (h c) -> p h c", c=HD),
                )
                nc.vector.memset(vv[:, :, HD:HD + 1], 1.0)

        # ---- query-half outer loop: projections, attention, y-projection ----
        def emit_y(hf, dcs):
            for dc in dcs:
                yps = [
                    pjpool.tile([128, QC], F32, tag="pj", name=f"yps{hf}_{dc}_{j}")
                    for j in range(NQC)
                ]
                for oc in range(NOC):
                    for j in range(NQC):
                        nc.tensor.matmul(
                            yps[j][:, :],
                            wo_sb[:, oc, dc * 128:(dc + 1) * 128],
                            att_sb[hf][:, oc, j * QC:(j + 1) * QC],
                            start=(oc == 0), stop=(oc == NOC - 1),
                        )
                for j in range(NQC):
                    y_sb = evpool.tile([128, QC], F32, tag="yev", name=f"yev{hf}_{dc}_{j}")
                    nc.vector.tensor_copy(out=y_sb[:, :], in_=yps[j][:, :])
                    nc.sync.dma_start(
                        out=y_d.ap()[dc * 128:(dc + 1) * 128,
                                     hf * HF + j * QC: hf * HF + (j + 1) * QC],
                        in_=y_sb[:, :],
                    )

        for hf in range(2):
            for hp in range(NOC):
                if hf == 0:
                    for pi, (w_sb, x_sb, b_sb, dst) in enumerate((
                        (wq_sb, xq_sb, bq_sb, qT_sb),
                        (wk_sb, xk_sb, bk_sb, kT_sb),
                    )):
                        for tp in range(NTQ // 2):
                            pps = [
                                pjpool.tile([128, QC], F32, tag="pj",
                                            name=f"pj{hp}_{pi}_{tp}_{j}")
                                for j in range(2)
                            ]
                            for ic in range(NI):
                                for j in range(2):
                                    tq = tp * 2 + j
                                    nc.tensor.matmul(
                                        pps[j][:, :],
                                        w_sb[:, ic, hp * 128:(hp + 1) * 128],
                                        x_sb[:, ic, tq * QC:(tq + 1) * QC],
                                        start=(ic == 0), stop=(ic == NI - 1),
                                    )
                            for j in range(2):
                                tq = tp * 2 + j
                                nc.vector.tensor_scalar_add(
                                    out=dst[:, hp, tq * QC:(tq + 1) * QC],
                                    in0=pps[j][:, :],
                                    scalar1=b_sb[:, hp:hp + 1],
                                )

                for hl in range(2):
                    h = 2 * hp + hl
                    off = hl * 64
                    vslice = slice(h * (HD + 1), (h + 1) * (HD + 1))
                    accs = [
                        accpool.tile([65, QC], F32, tag="acc", name=f"acc{h}_{hf}_{i}")
                        for i in range(NQC)
                    ]

                    def s2(e, kc):
                        for tq in range(NQC):
                            nc.tensor.matmul(
                                accs[tq][:, :],
                                v_sb[:, kc, vslice],
                                e[:, tq * QC:(tq + 1) * QC],
                                start=(kc == 0), stop=(kc == NKC - 1),
                            )

                    prev = None
                    for kc in range(NKC):
                        s1 = s1pool.tile([128, HF], F32, tag="s1", name=f"s1_{h}_{hf}_{kc}")
                        for tq in range(NQC):
                            nc.tensor.matmul(
                                s1[:, tq * QC:(tq + 1) * QC],
                                kT_sb[off:off + 64, hp, kc * 128:(kc + 1) * 128],
                                qT_sb[off:off + 64, hp,
                                      hf * HF + tq * QC: hf * HF + (tq + 1) * QC],
                                start=True, stop=True,
                            )
                        e = epool.tile([128, HF], BF16, tag="e", name=f"e{h}_{hf}_{kc}")
                        nc.scalar.activation(out=e[:, :], in_=s1[:, :], func=Exp)
                        if prev is not None:
                            s2(*prev)
                        prev = (e, kc)
                    s2(*prev)

                    # free the acc banks first, then normalize from the copies
                    asbs = []
                    for tq in range(NQC):
                        asb = npool.tile([65, QC], F32, tag="accsb", bufs=2,
                                         name=f"asb{h}_{hf}_{tq}")
                        nc.vector.tensor_copy(out=asb[:, :], in_=accs[tq][:, :])
                        asbs.append(asb)
                    for tq in range(NQC):
                        asb = asbs[tq]
                        rt = npool.tile([1, QC], F32, tag="rtmp", bufs=2,
                                        name=f"rt{h}_{hf}_{tq}")
                        nc.vector.tensor_copy(out=rt[:, :], in_=asb[64:65, :])
                        nc.vector.reciprocal_approx_fast(out=rt[:, :], in_=rt[:, :])
                        bc = npool.tile([64, QC], F32, tag="bcast", bufs=2,
                                        name=f"bc{h}_{hf}_{tq}")
                        nc.gpsimd.partition_broadcast(out_ap=bc[:, :], in_ap=rt[:, :])
                        nc.vector.tensor_tensor(
                            out=att_sb[hf][off:off + 64, hp, tq * QC:(tq + 1) * QC],
                            in0=asb[0:64, :],
                            in1=bc[:, :],
                            op=Mult,
                        )

                if hf == 1:
                    # spread half-0's output projection across half-1's
                    # attention stream (issued only after this head pair's
                    # attention so it never heads the in-order PE queue while
                    # waiting on the DVE normalize)
                    emit_y(0, range(hp * 2, hp * 2 + 2))

        emit_y(1, range(NDC))

    nc.compile()
    return nc


def make_in_maps(query, key, value, Wq, bq, Wk, bk, Wv, bv, Wo, bo):
    """Shard + lay out full inputs for the 8 cores: core = 2*n + g."""
    f16 = np.float16
    N = query.shape[0]
    per_g = {}
    for g in range(2):
        osl = slice(g * OC, (g + 1) * OC)
        per_g[g] = dict(
            WqT=np.ascontiguousarray(Wq[osl, :].T).astype(f16),
            WkT=np.ascontiguousarray(Wk[osl, :].T).astype(f16),
            WvT=np.ascontiguousarray(Wv[osl, :].T).astype(f16),
            WoT=np.ascontiguousarray(Wo[:, osl].T).astype(f16),
            bq=np.ascontiguousarray(bq[osl]).astype(np.float32),
            bk=np.ascontiguousarray(bk[osl]).astype(np.float32),
        )
    in_maps = []
    for n in range(N):
        xqT = np.ascontiguousarray(query[n].T).astype(f16)
        xkT = np.ascontiguousarray(key[n].T).astype(f16)
        xvT = np.ascontiguousarray(value[n].T).astype(f16)
        for g in range(2):
            m = dict(xqT=xqT, xkT=xkT, xvT=xvT)
            m.update(per_g[g])
            in_maps.append(m)
    return in_maps


_BUILT = None


def _get_built():
    global _BUILT
    if _BUILT is None:
        _BUILT = build_kernel(2048)
    return _BUILT


def kernel(query, key, value, Wq, bq, Wk, bk, Wv, bv, Wo, bo, _results=None):
    query = np.asarray(query, np.float32)
    key = np.asarray(key, np.float32)
    value = np.asarray(value, np.float32)
    Wq, bq = np.asarray(Wq, np.float32), np.asarray(bq, np.float32)
    Wk, bk = np.asarray(Wk, np.float32), np.asarray(bk, np.float32)
    Wv, bv = np.asarray(Wv, np.float32), np.asarray(bv, np.float32)
    Wo, bo = np.asarray(Wo, np.float32), np.asarray(bo, np.float32)

    N, S, _ = query.shape
    if _results is None:
        nc = _get_built()
        in_maps = make_in_maps(query, key, value, Wq, bq, Wk, bk, Wv, bv, Wo, bo)
        res = run_bass_kernel_spmd(nc, in_maps, list(range(N_CORES)))
        _results = res.results

    const = bv @ Wo.T + bo  # host-folded bias terms
    out = np.empty((N, S, D), np.float32)
    for n in range(N):
        yT = _results[2 * n]["yT"] + _results[2 * n + 1]["yT"]
        out[n] = yT.T + const
    return out


# revision 25
# speedup vs baseline: 1.0486x; 1.0156x over previous
"""Trainium2 Bass kernel for 16-head MHA (d_model=1024, batch 4, seq 2048).

Sharding: batch (4) x head-group (2) across 8 NeuronCores. Each core computes
one batch sample's attention for 8 of the 16 heads plus its partial output
projection; the host sums the two partial outputs per sample and adds the
bias terms.

Per-core dataflow (all matmul contractions run on the partition axis):
  q^T/k^T = WqT'.T @ x^T   (fp16, out [o, t] with heads on partitions)
  v       = x^T.T @ WvT    (fp16 matmul, bf16 store, out [t, o], with a ones
                            column appended per head for fused softmax rowsums)
  E^T     = exp(k^T_h.T @ q^T_h)  (scores accumulate f32 in PSUM, exp on ACT,
                                   bf16 store; no max-subtraction needed: the
                                   reference applies no 1/sqrt(d) scaling and
                                   scores stay < ~50, well within f32/bf16 range)
  att_h   = (V_h|1).T @ E^T_h     -> rows 0:64 raw attention, row 64 rowsum
  att^T   = att_h * recip(rowsum) (DVE + gpsimd broadcast, off critical path)
  y^T     = WoT'.T @ att^T        (fp32 out)

fp16 is used for the whole q/k/scores path: bf16's 8-bit mantissa gives score
errors ~0.05 which exp() amplifies to ~2e-2 output error; fp16 keeps it ~3e-3.

Loop structure: query-halves outermost so the output projection for half 0
overlaps the attention of half 1; projections pair two moving streams per
stationary load to hide weight-switch bubbles; the kc loop is
software-pipelined (S1(kc+1) issues before S2(kc)) so the PE never waits for
the ACT exp.
"""

from contextlib import ExitStack

import numpy as np

import concourse.bacc as bacc
import concourse.mybir as mybir
import concourse.tile as tile
from concourse.bass_utils import run_bass_kernel_spmd

F32 = mybir.dt.float32
F16 = mybir.dt.float16
BF16 = mybir.dt.bfloat16

D = 1024          # d_model
HD = 64           # head dim
NH_CORE = 8       # heads per core
OC = NH_CORE * HD # per-core q/k/v output dims (512)
N_CORES = 8
NI = D // 128     # contraction chunks for projections
NOC = OC // 128   # o-chunks (head pairs)
NDC = D // 128    # output-dim chunks for the final projection


def build_kernel(S=2048):
    nc = bacc.Bacc("TRN2", target_bir_lowering=False, debug=False)

    xq_d = nc.dram_tensor("xqT", (D, S), F16, kind="ExternalInput")
    xk_d = nc.dram_tensor("xkT", (D, S), F16, kind="ExternalInput")
    xv_d = nc.dram_tensor("xvT", (D, S), F16, kind="ExternalInput")
    wq_d = nc.dram_tensor("WqT", (D, OC), F16, kind="ExternalInput")
    wk_d = nc.dram_tensor("WkT", (D, OC), F16, kind="ExternalInput")
    wv_d = nc.dram_tensor("WvT", (D, OC), F16, kind="ExternalInput")
    wo_d = nc.dram_tensor("WoT", (OC, D), F16, kind="ExternalInput")
    bq_d = nc.dram_tensor("bq", (OC,), F32, kind="ExternalInput")
    bk_d = nc.dram_tensor("bk", (OC,), F32, kind="ExternalInput")
    y_d = nc.dram_tensor("yT", (D, S), F32, kind="ExternalOutput")

    NKC = S // 128            # key chunks
    HF = S // 2               # query-half size (exp/psum granularity)
    QC = min(512, HF)         # matmul moving size
    NQC = HF // QC
    NTQ = S // QC             # 512-col chunks over the full sequence
    VT = min(256, S)          # xv chunk width (2 stationary tiles per chunk)

    Exp = mybir.ActivationFunctionType.Exp
    Mult = mybir.AluOpType.mult

    with tile.TileContext(nc) as tc, ExitStack() as ctx:
        wpool = ctx.enter_context(tc.tile_pool(name="w", bufs=1))
        xpool = ctx.enter_context(tc.tile_pool(name="x", bufs=1))
        spool = ctx.enter_context(tc.tile_pool(name="seq", bufs=1))
        epool = ctx.enter_context(tc.tile_pool(name="e", bufs=4))
        evpool = ctx.enter_context(tc.tile_pool(name="ev", bufs=2))
        npool = ctx.enter_context(tc.tile_pool(name="nrm", bufs=2))
        pjpool = ctx.enter_context(tc.tile_pool(name="pj", bufs=2, space="PSUM"))
        s1pool = ctx.enter_context(tc.tile_pool(name="s1", bufs=2, space="PSUM"))
        accpool = ctx.enter_context(tc.tile_pool(name="acc", bufs=2, space="PSUM"))

        # ---- resident weights / biases / x slabs ----
        wq_sb = wpool.tile([128, NI, OC], F16, tag="wq")
        wk_sb = wpool.tile([128, NI, OC], F16, tag="wk")
        wv_sb = wpool.tile([128, NI, OC], F16, tag="wv")
        wo_sb = wpool.tile([128, NOC, D], F16, tag="wo")
        nc.sync.dma_start(out=wv_sb, in_=wv_d.ap().rearrange("(ic p) o -> p ic o", p=128))
        nc.sync.dma_start(out=wq_sb, in_=wq_d.ap().rearrange("(ic p) o -> p ic o", p=128))
        nc.sync.dma_start(out=wk_sb, in_=wk_d.ap().rearrange("(ic p) o -> p ic o", p=128))
        nc.sync.dma_start(out=wo_sb, in_=wo_d.ap().rearrange("(oc p) d -> p oc d", p=128))
        bq_sb = wpool.tile([128, NOC], F32, tag="bq")
        bk_sb = wpool.tile([128, NOC], F32, tag="bk")
        nc.sync.dma_start(out=bq_sb, in_=bq_d.ap().rearrange("(c p) -> p c", p=128))
        nc.sync.dma_start(out=bk_sb, in_=bk_d.ap().rearrange("(c p) -> p c", p=128))

        xq_sb = xpool.tile([128, NI, S], F16, tag="xq")
        xk_sb = xpool.tile([128, NI, S], F16, tag="xk")
        nc.sync.dma_start(out=xq_sb, in_=xq_d.ap().rearrange("(ic p) t -> p ic t", p=128))
        nc.sync.dma_start(out=xk_sb, in_=xk_d.ap().rearrange("(ic p) t -> p ic t", p=128))

        # ---- per-sequence slabs ----
        v_sb = spool.tile([128, NKC, NH_CORE * (HD + 1)], BF16, tag="v")
        qT_sb = spool.tile([128, NOC, S], F16, tag="qT")
        kT_sb = spool.tile([128, NOC, S], F16, tag="kT")
        att_sb = [
            spool.tile([128, NOC, HF], F16, tag=f"att{hf}", name=f"att{hf}")
            for hf in range(2)
        ]

        # ---- phase 0: v projection (v[t, o] layout, 65-stride + ones col) ----
        # xv streams in [128, VT] chunks so the big xq/xk DMAs overlap phase 0
        NVG = S // VT
        NVT = VT // 128
        for tcg in range(NVG):
            xvc = [
                xpool.tile([128, VT], F16, tag="xvc", bufs=16, name=f"xvc{tcg}_{ic}")
                for ic in range(NI)
            ]
            for ic in range(NI):
                nc.sync.dma_start(
                    out=xvc[ic],
                    in_=xv_d.ap()[ic * 128:(ic + 1) * 128, tcg * VT:(tcg + 1) * VT],
                )
            vps = [
                pjpool.tile([128, OC], F32, tag="pj", name=f"vps{tcg}_{j}")
                for j in range(NVT)
            ]
            for ic in range(NI):
                for j in range(NVT):
                    nc.tensor.matmul(
                        vps[j][:, :],
                        xvc[ic][:, j * 128:(j + 1) * 128],
                        wv_sb[:, ic, :],
                        start=(ic == 0), stop=(ic == NI - 1),
                    )
            for j in range(NVT):
                tci = tcg * NVT + j
                vv = v_sb[:, tci, :].rearrange("p (h c) -> p h c", h=NH_CORE)
                nc.vector.tensor_copy(
                    out=vv[:, :, 0:HD],
                    in_=vps[j][:, :].rearrange("p (h c) -> p h c", c=HD),
                )
                nc.vector.memset(vv[:, :, HD:HD + 1], 1.0)

        # ---- query-half outer loop: projections, attention, y-projection ----
        def emit_y(hf, dcs):
            for dc in dcs:
                yps = [
                    pjpool.tile([128, QC], F32, tag="pj", name=f"yps{hf}_{dc}_{j}")
                    for j in range(NQC)
                ]
                for oc in range(NOC):
                    for j in range(NQC):
                        nc.tensor.matmul(
                            yps[j][:, :],
                            wo_sb[:, oc, dc * 128:(dc + 1) * 128],
                            att_sb[hf][:, oc, j * QC:(j + 1) * QC],
                            start=(oc == 0), stop=(oc == NOC - 1),
                        )
                for j in range(NQC):
                    y_sb = evpool.tile([128, QC], F32, tag="yev", name=f"yev{hf}_{dc}_{j}")
                    nc.vector.tensor_copy(out=y_sb[:, :], in_=yps[j][:, :])
                    nc.sync.dma_start(
                        out=y_d.ap()[dc * 128:(dc + 1) * 128,
                                     hf * HF + j * QC: hf * HF + (j + 1) * QC],
                        in_=y_sb[:, :],
                    )

        for hf in range(2):
            for hp in range(NOC):
                if hf == 0:
                    for pi, (w_sb, x_sb, b_sb, dst) in enumerate((
                        (wq_sb, xq_sb, bq_sb, qT_sb),
                        (wk_sb, xk_sb, bk_sb, kT_sb),
                    )):
                        for tp in range(NTQ // 2):
                            pps = [
                                pjpool.tile([128, QC], F32, tag="pj",
                                            name=f"pj{hp}_{pi}_{tp}_{j}")
                                for j in range(2)
                            ]
                            for ic in range(NI):
                                for j in range(2):
                                    tq = tp * 2 + j
                                    nc.tensor.matmul(
                                        pps[j][:, :],
                                        w_sb[:, ic, hp * 128:(hp + 1) * 128],
                                        x_sb[:, ic, tq * QC:(tq + 1) * QC],
                                        start=(ic == 0), stop=(ic == NI - 1),
                                    )
                            for j in range(2):
                                tq = tp * 2 + j
                                nc.vector.tensor_scalar_add(
                                    out=dst[:, hp, tq * QC:(tq + 1) * QC],
                                    in0=pps[j][:, :],
                                    scalar1=b_sb[:, hp:hp + 1],
                                )

                for hl in range(2):
                    h = 2 * hp + hl
                    off = hl * 64
                    vslice = slice(h * (HD + 1), (h + 1) * (HD + 1))
                    accs = [
                        accpool.tile([65, QC], F32, tag="acc", name=f"acc{h}_{hf}_{i}")
                        for i in range(NQC)
                    ]

                    def s2(e, kc):
                        for tq in range(NQC):
                            nc.tensor.matmul(
                                accs[tq][:, :],
                                v_sb[:, kc, vslice],
                                e[:, tq * QC:(tq + 1) * QC],
                                start=(kc == 0), stop=(kc == NKC - 1),
                            )

                    # depth-2 software pipeline: S2 trails the exp by two
                    # chunks so it never heads the PE queue waiting on ACT
                    pend = []
                    for kc in range(NKC):
                        s1 = s1pool.tile([128, HF], F32, tag="s1", name=f"s1_{h}_{hf}_{kc}")
                        for tq in range(NQC):
                            nc.tensor.matmul(
                                s1[:, tq * QC:(tq + 1) * QC],
                                kT_sb[off:off + 64, hp, kc * 128:(kc + 1) * 128],
                                qT_sb[off:off + 64, hp,
                                      hf * HF + tq * QC: hf * HF + (tq + 1) * QC],
                                start=True, stop=True,
                            )
                        e = epool.tile([128, HF], BF16, tag="e", name=f"e{h}_{hf}_{kc}")
                        nc.scalar.activation(out=e[:, :], in_=s1[:, :], func=Exp)
                        pend.append((e, kc))
                        if len(pend) > 2:
                            s2(*pend.pop(0))
                    for item in pend:
                        s2(*item)

                    # free the acc banks first, then normalize from the copies
                    asbs = []
                    for tq in range(NQC):
                        asb = npool.tile([65, QC], F32, tag="accsb", bufs=2,
                                         name=f"asb{h}_{hf}_{tq}")
                        nc.vector.tensor_copy(out=asb[:, :], in_=accs[tq][:, :])
                        asbs.append(asb)
                    for tq in range(NQC):
                        asb = asbs[tq]
                        rt = npool.tile([1, QC], F32, tag="rtmp", bufs=2,
                                        name=f"rt{h}_{hf}_{tq}")
                        nc.vector.tensor_copy(out=rt[:, :], in_=asb[64:65, :])
                        nc.vector.reciprocal_approx_fast(out=rt[:, :], in_=rt[:, :])
                        bc = npool.tile([64, QC], F32, tag="bcast", bufs=2,
                                        name=f"bc{h}_{hf}_{tq}")
                        nc.gpsimd.partition_broadcast(out_ap=bc[:, :], in_ap=rt[:, :])
                        nc.vector.tensor_tensor(
                            out=att_sb[hf][off:off + 64, hp, tq * QC:(tq + 1) * QC],
                            in0=asb[0:64, :],
                            in1=bc[:, :],
                            op=Mult,
                        )

                if hf == 1:
                    # spread half-0's output projection across half-1's
                    # attention stream (issued only after this head pair's
                    # attention so it never heads the in-order PE queue while
                    # waiting on the DVE normalize)
                    emit_y(0, range(hp * 2, hp * 2 + 2))

        emit_y(1, range(NDC))

    nc.compile()
    return nc


def make_in_maps(query, key, value, Wq, bq, Wk, bk, Wv, bv, Wo, bo):
    """Shard + lay out full inputs for the 8 cores: core = 2*n + g."""
    f16 = np.float16
    N = query.shape[0]
    per_g = {}
    for g in range(2):
        osl = slice(g * OC, (g + 1) * OC)
        per_g[g] = dict(
            WqT=np.ascontiguousarray(Wq[osl, :].T).astype(f16),
            WkT=np.ascontiguousarray(Wk[osl, :].T).astype(f16),
            WvT=np.ascontiguousarray(Wv[osl, :].T).astype(f16),
            WoT=np.ascontiguousarray(Wo[:, osl].T).astype(f16),
            bq=np.ascontiguousarray(bq[osl]).astype(np.float32),
            bk=np.ascontiguousarray(bk[osl]).astype(np.float32),
        )
    in_maps = []
    for n in range(N):
        xqT = np.ascontiguousarray(query[n].T).astype(f16)
        xkT = np.ascontiguousarray(key[n].T).astype(f16)
        xvT = np.ascontiguousarray(value[n].T).astype(f16)
        for g in range(2):
            m = dict(xqT=xqT, xkT=xkT, xvT=xvT)
            m.update(per_g[g])
            in_maps.append(m)
    return in_maps


_BUILT = None


def _get_built():
    global _BUILT
    if _BUILT is None:
        _BUILT = build_kernel(2048)
    return _BUILT


def kernel(query, key, value, Wq, bq, Wk, bk, Wv, bv, Wo, bo, _results=None):
    query = np.asarray(query, np.float32)
    key = np.asarray(key, np.float32)
    value = np.asarray(value, np.float32)
    Wq, bq = np.asarray(Wq, np.float32), np.asarray(bq, np.float32)
    Wk, bk = np.asarray(Wk, np.float32), np.asarray(bk, np.float32)
    Wv, bv = np.asarray(Wv, np.float32), np.asarray(bv, np.float32)
    Wo, bo = np.asarray(Wo, np.float32), np.asarray(bo, np.float32)

    N, S, _ = query.shape
    if _results is None:
        nc = _get_built()
        in_maps = make_in_maps(query, key, value, Wq, bq, Wk, bk, Wv, bv, Wo, bo)
        res = run_bass_kernel_spmd(nc, in_maps, list(range(N_CORES)))
        _results = res.results

    const = bv @ Wo.T + bo  # host-folded bias terms
    out = np.empty((N, S, D), np.float32)
    for n in range(N):
        yT = _results[2 * n]["yT"] + _results[2 * n + 1]["yT"]
        out[n] = yT.T + const
    return out
